# revision 16
# baseline (speedup 1.0000x reference)
"""Trainium2 Bass kernel for a 2-layer complex-gated GRU.

Problem (hardcoded):
  B=128, T=128, IN=256, H=2048, OUT=64, fp32.
  reference: 2 stacked complex GRU cells scanned over T, then a complex FC.

Strategy: 8-way tensor-parallel over the complex gate output dim (each core
owns 128 of the 1024 complex outputs per gate => 256 rows of the 2048-row
real-valued hidden state). Everything lives transposed ([features, batch]).

Each complex linear uses the 3-multiplication (Gauss) form:
  t1 = Wr   @ (cr+ci)
  t2 = (Wr+Wi) @ ci
  t3 = (Wi-Wr) @ cr
  out_r = t1 - t2,  out_i = t1 + t3
which is 25% fewer matmul FLOPs than the merged 2Kx2M real form. t1/t2/t3
accumulate in separate PSUM column slots; a DVE sub/add pair combines them,
then ScalarE applies sigmoid/tanh with the bias.

Per step, each cell needs two 8-core all-gathers: tmp = r (.) h (computed on
the owner core from its fp32 state shard BEFORE the gather - no full-width
multiply after), and h'. The two cells are software-pipelined (cell0 at t
with cell1 at t-1) so collective latency hides under the other cell's
matmuls. The r-gate is computed before the z-gate so its AG dispatches
early; the z-gate matmuls run under the AG. Matmul operands are bf16,
accumulation fp32 in PSUM, state update in fp32 from the core-local shard.
"""

import numpy as np
import ml_dtypes

B, T, IN, H, OUT = 128, 128, 256, 2048, 64
NCORES = 8
M = H // 2  # 1024 complex gate outputs per gate
SH = M // NCORES  # 128 complex outputs per core per gate
N0 = (IN + H) // 2  # 1152 complex contraction length (layer 0)
N1 = (H + H) // 2  # 2048 complex contraction length (layer 1)
NK0 = N0 // 128  # 9
NK1 = N1 // 128  # 16
NHT = H // 128  # 16 tiles of the hidden state

BF16 = ml_dtypes.bfloat16


def _sbuf_layout(w, ncols=128):
    """[K, ncols] -> [128, (K//128)*ncols] with k-tile k at cols [k*nc:(k+1)*nc]."""
    K = w.shape[0]
    nk = K // 128
    return np.ascontiguousarray(
        w.reshape(nk, 128, ncols).transpose(1, 0, 2).reshape(128, nk * ncols)
    )


def _gate_w(Wr, Wi, g, s):
    """Stationary weights for gate g, core s: [128, 3*nk*128] bf16.

    Slots (each nk k-tiles of [128k x 128out]):
      t1 = Wr (vs summed input), t2 = Wr+Wi (vs imag), t3 = Wi-Wr (vs real).
    Contraction rows are the natural complex input order (x first, then h
    complex components 0..1023 - which matches the gathered storage layout).
    """
    sl = slice(s * SH, (s + 1) * SH)
    a1 = Wr[g][sl]  # [128, n]
    a2 = (Wr[g] + Wi[g])[sl]
    a3 = (Wi[g] - Wr[g])[sl]
    blocks = [_sbuf_layout(a.T) for a in (a1, a2, a3)]
    return np.concatenate(blocks, axis=1)


def host_prep(inputs):
    """Build per-core in_maps (numpy) from the full problem inputs."""
    x = np.asarray(inputs["x"], np.float32)
    # xt[t] = [xr^T | xi^T | (xr+xi)^T], each [128, B]
    xr = x[:, :, :128]  # [B, T, 128]
    xi = x[:, :, 128:]
    xs = xr + xi
    xt = np.stack(
        [
            np.transpose(xr, (1, 2, 0)),  # [T, 128, B]
            np.transpose(xi, (1, 2, 0)),
            np.transpose(xs, (1, 2, 0)),
        ],
        axis=2,
    ).reshape(T, 128, 3 * B)
    xt = np.ascontiguousarray(xt).astype(BF16)

    # FC: out = [out_r | out_i] = h1c @ Wfc_big.T + [fcbr | fcbi]
    fcWr = np.asarray(inputs["fcWr"], np.float32)
    fcWi = np.asarray(inputs["fcWi"], np.float32)
    Wfc_big = np.block([[fcWr, -fcWi], [fcWi, fcWr]])  # [64, 2048] natural cols
    # storage layout: row p of h1f-tiles: shard s=p//256, q=p%256;
    # q<128 -> real comp s*128+q, q>=128 -> imag comp s*128+(q-128)
    p = np.arange(H)
    s_ = p // 256
    q = p % 256
    comp = s_ * 128 + (q % 128)
    h_perm = np.where(q < 128, comp, M + comp)
    Wfc_perm = Wfc_big[:, h_perm]
    wfc = np.ascontiguousarray(
        Wfc_perm.T.reshape(NHT, 128, OUT).transpose(1, 0, 2).reshape(128, NHT * OUT)
    ).astype(BF16)
    fcb = np.concatenate([inputs["fcbr"], inputs["fcbi"]]).astype(np.float32)
    fcbias = np.ascontiguousarray(np.broadcast_to(fcb, (128, OUT))).astype(np.float32)

    W0r = np.asarray(inputs["W0r"], np.float32)
    W0i = np.asarray(inputs["W0i"], np.float32)
    W1r = np.asarray(inputs["W1r"], np.float32)
    W1i = np.asarray(inputs["W1i"], np.float32)

    in_maps = []
    for s in range(NCORES):
        sl = slice(s * SH, (s + 1) * SH)
        bias = np.stack(
            [
                inputs["b0r"][0][sl], inputs["b0i"][0][sl],
                inputs["b0r"][1][sl], inputs["b0i"][1][sl],
                inputs["b0r"][2][sl], inputs["b0i"][2][sl],
                inputs["b1r"][0][sl], inputs["b1i"][0][sl],
                inputs["b1r"][1][sl], inputs["b1i"][1][sl],
                inputs["b1r"][2][sl], inputs["b1i"][2][sl],
            ],
            axis=1,
        ).astype(np.float32)  # [128, 12]
        in_maps.append(
            dict(
                xt=xt,
                wz0=_gate_w(W0r, W0i, 0, s).astype(BF16),
                wr0=_gate_w(W0r, W0i, 1, s).astype(BF16),
                wh0=_gate_w(W0r, W0i, 2, s).astype(BF16),
                wz1=_gate_w(W1r, W1i, 0, s).astype(BF16),
                wr1=_gate_w(W1r, W1i, 1, s).astype(BF16),
                wh1=_gate_w(W1r, W1i, 2, s).astype(BF16),
                wfc=wfc,
                bias=np.ascontiguousarray(bias),
                fcbias=fcbias,
            )
        )
    return in_maps


# ---------------------------------------------------------------------------
# numpy emulation of the sharded algorithm (host-side validation only)
# ---------------------------------------------------------------------------


def numpy_sharded_reference(inputs, t_steps=T):
    """Emulates the bass kernel's math in fp32 (no bf16 rounding)."""
    x = np.asarray(inputs["x"], np.float32)

    def sig(v):
        return 1.0 / (1.0 + np.exp(-v))

    def clin3(cr, ci, Wr, Wi, br, bi):
        # [n, B] inputs (natural complex order); Wr/Wi [1024, n]
        t1 = Wr @ (cr + ci)
        t2 = (Wr + Wi) @ ci
        t3 = (Wi - Wr) @ cr
        return t1 - t2 + br[:, None], t1 + t3 + bi[:, None]

    W0r = np.asarray(inputs["W0r"], np.float32)
    W0i = np.asarray(inputs["W0i"], np.float32)
    W1r = np.asarray(inputs["W1r"], np.float32)
    W1i = np.asarray(inputs["W1i"], np.float32)

    hr = [np.zeros((M, B), np.float32) for _ in range(2)]
    hi = [np.zeros((M, B), np.float32) for _ in range(2)]
    for t in range(t_steps):
        xr = x[:, t, :128].T
        xi = x[:, t, 128:].T
        for layer in range(2):
            Wr = (W0r, W1r)[layer]
            Wi = (W0i, W1i)[layer]
            br = (inputs["b0r"], inputs["b1r"])[layer]
            bi = (inputs["b0i"], inputs["b1i"])[layer]
            if layer == 0:
                cr = np.concatenate([xr, hr[0]], axis=0)
                ci = np.concatenate([xi, hi[0]], axis=0)
            else:
                cr = np.concatenate([hr[0], hr[1]], axis=0)
                ci = np.concatenate([hi[0], hi[1]], axis=0)
            zr, zi = clin3(cr, ci, Wr[0], Wi[0], br[0], bi[0])
            z_r, z_i = sig(zr), sig(zi)
            rr, ri = clin3(cr, ci, Wr[1], Wi[1], br[1], bi[1])
            r_r, r_i = sig(rr), sig(ri)
            tr = r_r * hr[layer]
            ti = r_i * hi[layer]
            if layer == 0:
                cr2 = np.concatenate([xr, tr], axis=0)
                ci2 = np.concatenate([xi, ti], axis=0)
            else:
                cr2 = np.concatenate([hr[0], tr], axis=0)
                ci2 = np.concatenate([hi[0], ti], axis=0)
            hhr, hhi = clin3(cr2, ci2, Wr[2], Wi[2], br[2], bi[2])
            hh_r, hh_i = np.tanh(hhr), np.tanh(hhi)
            hr[layer] = (1.0 - z_r) * hr[layer] + z_r * hh_r
            hi[layer] = (1.0 - z_i) * hi[layer] + z_i * hh_i
    out_r = np.asarray(inputs["fcWr"], np.float32) @ hr[1] \
        - np.asarray(inputs["fcWi"], np.float32) @ hi[1] \
        + np.asarray(inputs["fcbr"], np.float32)[:, None]
    out_i = np.asarray(inputs["fcWr"], np.float32) @ hi[1] \
        + np.asarray(inputs["fcWi"], np.float32) @ hr[1] \
        + np.asarray(inputs["fcbi"], np.float32)[:, None]
    return np.concatenate([out_r, out_i], axis=0).T


# ---------------------------------------------------------------------------
# bass kernel
# ---------------------------------------------------------------------------


def build_kernel(t_steps=T, no_collectives=False):
    import concourse.bacc as bacc
    import concourse.mybir as mybir
    import concourse.tile as tile

    fp32 = mybir.dt.float32
    bf16 = mybir.dt.bfloat16
    AF = mybir.ActivationFunctionType

    nc = bacc.Bacc(
        "TRN2", target_bir_lowering=False, debug=False, num_devices=NCORES
    )

    d_xt = nc.dram_tensor("xt", [T, 128, 3 * B], bf16, kind="ExternalInput")
    d_wz0 = nc.dram_tensor("wz0", [128, 3 * NK0 * 128], bf16, kind="ExternalInput")
    d_wr0 = nc.dram_tensor("wr0", [128, 3 * NK0 * 128], bf16, kind="ExternalInput")
    d_wh0 = nc.dram_tensor("wh0", [128, 3 * NK0 * 128], bf16, kind="ExternalInput")
    d_wz1 = nc.dram_tensor("wz1", [128, 3 * NK1 * 128], bf16, kind="ExternalInput")
    d_wr1 = nc.dram_tensor("wr1", [128, 3 * NK1 * 128], bf16, kind="ExternalInput")
    d_wh1 = nc.dram_tensor("wh1", [128, 3 * NK1 * 128], bf16, kind="ExternalInput")
    d_wfc = nc.dram_tensor("wfc", [128, NHT * OUT], bf16, kind="ExternalInput")
    d_bias = nc.dram_tensor("bias", [128, 12], fp32, kind="ExternalInput")
    d_fcbias = nc.dram_tensor("fcbias", [128, OUT], fp32, kind="ExternalInput")
    d_out = nc.dram_tensor("out", [B, OUT], fp32, kind="ExternalOutput")

    RG = [list(range(NCORES))]

    with tile.TileContext(nc) as tc:
        with (
            tc.tile_pool(name="wpool", bufs=1) as wpool,
            tc.tile_pool(name="state", bufs=1) as spool,
            tc.tile_pool(name="work", bufs=2) as work,
            tc.tile_pool(name="xpool", bufs=3) as xpool,
            tc.tile_pool(name="pzr", bufs=1, space="PSUM") as pzr_pool,
            tc.tile_pool(name="ph", bufs=1, space="PSUM") as ph_pool,
            tc.tile_pool(name="dram", bufs=2, space="DRAM") as dram,
        ):
            # --- persistent weights -> SBUF ---
            wz0 = wpool.tile([128, 3 * NK0 * 128], bf16)
            wr0 = wpool.tile([128, 3 * NK0 * 128], bf16)
            wh0 = wpool.tile([128, 3 * NK0 * 128], bf16)
            wz1 = wpool.tile([128, 3 * NK1 * 128], bf16)
            wr1 = wpool.tile([128, 3 * NK1 * 128], bf16)
            wh1 = wpool.tile([128, 3 * NK1 * 128], bf16)
            wfc = wpool.tile([128, NHT * OUT], bf16)
            bias = wpool.tile([128, 12], fp32)
            fcbias = wpool.tile([128, OUT], fp32)
            for dst, src in [
                (wz0, d_wz0), (wr0, d_wr0), (wh0, d_wh0),
                (wz1, d_wz1), (wr1, d_wr1), (wh1, d_wh1),
                (wfc, d_wfc), (bias, d_bias), (fcbias, d_fcbias),
            ]:
                nc.sync.dma_start(dst[:], src[:])

            def mm_slot(psum_ap, wsb, nk, slot, rhs_entries):
                """Accumulate one Gauss slot into psum_ap ([128, 128]).

                wsb: [128, 3*nk*128] stationary; slot in {0,1,2}; rhs_entries =
                [(k, rhs_ap)] with k the k-tile index within the slot."""
                last = len(rhs_entries) - 1
                base = slot * nk * 128
                for idx, (kg, rap) in enumerate(rhs_entries):
                    nc.tensor.matmul(
                        psum_ap,
                        wsb[:, base + kg * 128 : base + (kg + 1) * 128],
                        rap,
                        start=(idx == 0),
                        stop=(idx == last),
                    )

            def h_r_tiles(hf, base):
                # real-part k-tiles of a gathered state (storage layout)
                return [(base + i, hf[:, i * 256 : i * 256 + 128]) for i in range(8)]

            def h_i_tiles(hf, base):
                return [
                    (base + i, hf[:, i * 256 + 128 : i * 256 + 256]) for i in range(8)
                ]

            def hs_tiles(hs, base):
                # summed (r+i) k-tiles [128, 1024]
                return [(base + i, hs[:, i * 128 : (i + 1) * 128]) for i in range(8)]

            # state buffers (python-managed ping-pong)
            h0f = [None, None]  # full h0 (bf16, [128, H]) storage layout
            h1f = [None, None]
            hs0 = [None, None]  # summed h0 (r+i) [128, 1024] bf16
            hs1 = [None, None]
            h0loc = [None, None]  # own fp32 shard [128, 256]
            h1loc = [None, None]

            def new_state_tiles(i):
                h0f[i] = spool.tile([128, H], bf16, name=f"h0f{i}")
                h1f[i] = spool.tile([128, H], bf16, name=f"h1f{i}")
                hs0[i] = spool.tile([128, M], bf16, name=f"hs0{i}")
                hs1[i] = spool.tile([128, M], bf16, name=f"hs1{i}")
                h0loc[i] = spool.tile([128, 256], fp32, name=f"h0loc{i}")
                h1loc[i] = spool.tile([128, 256], fp32, name=f"h1loc{i}")

            new_state_tiles(0)
            new_state_tiles(1)

            def sum_ri(dst, hf):
                # dst[128, 1024] = hf real cols + imag cols (strided over shards)
                nc.vector.tensor_add(
                    dst[:].rearrange("p (s c) -> p s c", c=128),
                    hf[:].rearrange("p (s two c) -> p s two c", two=2, c=128)[
                        :, :, 0, :
                    ],
                    hf[:].rearrange("p (s two c) -> p s two c", two=2, c=128)[
                        :, :, 1, :
                    ],
                )

            def emit_gate(w, nk, e1, e2, e3, pa_t1, pb, out_t, bcol, af, tag):
                """One Gauss gate: t2 MMs (bank b), t1 MMs (bank a, then an
                early ScalarE copy to SBUF that overlaps t3), t3 MMs (bank b),
                DVE combine, ScalarE activation.

                pa_t1: [128,128] psum AP (t1); pb: [128,256] psum AP (t2|t3)."""
                mm_slot(pb[:, 0:128], w, nk, 1, e2)  # t2
                mm_slot(pa_t1, w, nk, 0, e1)  # t1
                t1c = work.tile([128, 128], fp32, name=f"t1c_{tag}")
                nc.scalar.copy(t1c[:], pa_t1)
                mm_slot(pb[:, 128:256], w, nk, 2, e3)  # t3
                pq = work.tile([128, 256], fp32, name=f"pq_{tag}")
                nc.vector.tensor_sub(pq[:, 0:128], t1c[:], pb[:, 0:128])
                nc.vector.tensor_add(pq[:, 128:256], t1c[:], pb[:, 128:256])
                nc.scalar.activation(
                    out_t[:, 0:128], pq[:, 0:128], af,
                    bias=bias[:, bcol : bcol + 1],
                )
                nc.scalar.activation(
                    out_t[:, 128:256], pq[:, 128:256], af,
                    bias=bias[:, bcol + 1 : bcol + 2],
                )

            def ag_shard(shard_bf, tag):
                """DMA shard [128, 256] -> DRAM [256, 128], AllGather, return
                the gathered DRAM tile [H, 128]."""
                agin = dram.tile([256, B], bf16, name=f"agin_{tag}")
                agout = dram.tile(
                    [H, B], bf16, name=f"agout_{tag}",
                    addr_space="Local" if no_collectives else "Shared",
                )
                nc.scalar.dma_start(
                    agin[:].rearrange("(a p) b -> p a b", p=128),
                    shard_bf[:].rearrange("p (a b) -> p a b", b=B),
                )
                if no_collectives:
                    for s in range(NCORES):
                        nc.sync.dma_start(
                            agout[s * 256 : (s + 1) * 256, :], agin[:]
                        )
                else:
                    nc.gpsimd.collective_compute(
                        "AllGather",
                        mybir.AluOpType.bypass,
                        replica_groups=RG,
                        ins=[agin[:]],
                        outs=[agout[:]],
                    )
                return agout

            def dma_gather_in(hf, agout):
                half = NHT // 2
                for i, eng in enumerate((nc.sync, nc.scalar)):
                    eng.dma_start(
                        hf[:, i * half * 128 : (i + 1) * half * 128].rearrange(
                            "p (n b) -> p n b", b=B
                        ),
                        agout[i * half * 128 : (i + 1) * half * 128, :].rearrange(
                            "(n p) b -> p n b", p=128
                        ),
                    )

            def update_pre(loc_prev, z_t, tag):
                """Off-critical-path part of h' = (1-z)h + z hh = (h - z h) + z hh."""
                u = work.tile([128, 256], fp32, name=f"upd_u_{tag}")
                w_ = work.tile([128, 256], fp32, name=f"upd_w_{tag}")
                nc.vector.tensor_mul(u[:], z_t[:], loc_prev[:])
                nc.vector.tensor_sub(w_[:], loc_prev[:], u[:])
                return w_

            def state_update(w_, loc_new, z_t, hh_t, first):
                if first:
                    nc.vector.tensor_mul(loc_new[:], z_t[:], hh_t[:])
                else:
                    v = work.tile([128, 256], fp32, name="upd_v")
                    nc.vector.tensor_mul(v[:], z_t[:], hh_t[:])
                    nc.vector.tensor_add(loc_new[:], w_[:], v[:])

            # ---------------- main pipelined loop ----------------
            # slot t: cell0(t) interleaved with cell1(t-1)
            pend_ag_h0 = None  # AG of h0'(t-1), lands in h0f[cur]
            pend_ag_h1 = None  # AG of h1'(t-2), lands in h1f[cur]

            for t in range(t_steps + 1):
                cur = t % 2
                prv = 1 - cur

                if pend_ag_h0 is not None:
                    dma_gather_in(h0f[cur], pend_ag_h0)
                    pend_ag_h0 = None
                    sum_ri(hs0[cur], h0f[cur])
                if pend_ag_h1 is not None:
                    dma_gather_in(h1f[cur], pend_ag_h1)
                    pend_ag_h1 = None
                    sum_ri(hs1[cur], h1f[cur])

                # psum tiles for this slot:
                #   pt1  [128,512]: z0.t1 | r0.t1 | z1.t1 | r1.t1   (1 bank)
                #   pb_* [128,256]: t2 | t3 per gate               (1 bank each)
                #   pth  [128,256]: c0.h.t1 | c1.h.t1              (1 bank)
                pt1 = pzr_pool.tile([128, 512], fp32, name="pt1")
                pth = ph_pool.tile([128, 256], fp32, name="pth")

                ag_t0 = None
                z0_t = None
                if t < t_steps:
                    # ---- Phase A: cell0(t) r gate then z gate ----
                    xt_t = xpool.tile([128, 3 * B], bf16, name="xt_t")
                    nc.sync.dma_start(xt_t[:], d_xt[t])
                    e1, e2, e3 = (
                        [(0, xt_t[:, 256:384])],
                        [(0, xt_t[:, 128:256])],
                        [(0, xt_t[:, 0:128])],
                    )
                    if t > 0:
                        e1 = e1 + hs_tiles(hs0[cur], 1)
                        e2 = e2 + h_i_tiles(h0f[cur], 1)
                        e3 = e3 + h_r_tiles(h0f[cur], 1)
                    pb_r0 = pzr_pool.tile([128, 256], fp32, name="pb_r0")
                    r0_t = work.tile([128, 256], fp32, name="r0_t")
                    emit_gate(
                        wr0, NK0, e1, e2, e3, pt1[:, 128:256], pb_r0, r0_t,
                        2, AF.Sigmoid, "r0",
                    )
                    tmp0_sh = work.tile([128, 256], bf16, name="tmp0_sh")
                    if t > 0:
                        nc.vector.tensor_mul(tmp0_sh[:], r0_t[:], h0loc[prv][:])
                        ag_t0 = ag_shard(tmp0_sh, "t0")
                    pb_z0 = pzr_pool.tile([128, 256], fp32, name="pb_z0")
                    z0_t = work.tile([128, 256], fp32, name="z0_t")
                    emit_gate(
                        wz0, NK0, e1, e2, e3, pt1[:, 0:128], pb_z0, z0_t,
                        0, AF.Sigmoid, "z0",
                    )
                    if t > 0:
                        w0_upd = update_pre(h0loc[prv], z0_t, "h0")

                z1_t = None
                if 1 <= t:
                    # ---- Phase B (r part): cell1(t-1) r gate ----
                    e1b = hs_tiles(hs0[cur], 0)
                    e2b = h_i_tiles(h0f[cur], 0)
                    e3b = h_r_tiles(h0f[cur], 0)
                    if t > 1:
                        e1b = e1b + hs_tiles(hs1[cur], 8)
                        e2b = e2b + h_i_tiles(h1f[cur], 8)
                        e3b = e3b + h_r_tiles(h1f[cur], 8)
                    pb_r1 = pzr_pool.tile([128, 256], fp32, name="pb_r1")
                    r1_t = work.tile([128, 256], fp32, name="r1_t")
                    emit_gate(
                        wr1, NK1, e1b, e2b, e3b, pt1[:, 384:512], pb_r1, r1_t,
                        8, AF.Sigmoid, "r1",
                    )
                    tmp1_sh = work.tile([128, 256], bf16, name="tmp1_sh")
                    ag_t1 = None
                    if t > 1:
                        nc.vector.tensor_mul(tmp1_sh[:], r1_t[:], h1loc[prv][:])
                        ag_t1 = ag_shard(tmp1_sh, "t1")
                    # first half of the z1 gate here: these matmuls fill the
                    # PE gap while cell0's tmp gather is still in flight
                    pb_z1 = pzr_pool.tile([128, 256], fp32, name="pb_z1")
                    mm_slot(pb_z1[:, 0:128], wz1, NK1, 1, e2b)  # z1.t2
                    mm_slot(pt1[:, 256:384], wz1, NK1, 0, e1b)  # z1.t1
                    t1c_z1 = work.tile([128, 128], fp32, name="t1c_z1")
                    nc.scalar.copy(t1c_z1[:], pt1[:, 256:384])

                if t < t_steps:
                    # ---- Phase C: cell0(t) candidate + update ----
                    e1, e2, e3 = (
                        [(0, xt_t[:, 256:384])],
                        [(0, xt_t[:, 128:256])],
                        [(0, xt_t[:, 0:128])],
                    )
                    if t > 0:
                        tmp0f = work.tile([128, H], bf16, name="tmp0f")
                        dma_gather_in(tmp0f, ag_t0)
                        ts0 = work.tile([128, M], bf16, name="ts0")
                        sum_ri(ts0, tmp0f)
                        e1 = e1 + hs_tiles(ts0, 1)
                        e2 = e2 + h_i_tiles(tmp0f, 1)
                        e3 = e3 + h_r_tiles(tmp0f, 1)
                    pb_h0 = ph_pool.tile([128, 256], fp32, name="pb_h0")
                    hh0_t = work.tile([128, 256], fp32, name="hh0_t")
                    emit_gate(
                        wh0, NK0, e1, e2, e3, pth[:, 0:128], pb_h0, hh0_t,
                        4, AF.Tanh, "h0",
                    )
                    state_update(
                        w0_upd if t > 0 else None,
                        h0loc[cur], z0_t, hh0_t, first=(t == 0),
                    )
                    h0_sh = work.tile([128, 256], bf16, name="h0_sh")
                    nc.vector.tensor_copy(h0_sh[:], h0loc[cur][:])
                    pend_ag_h0 = ag_shard(h0_sh, "h0")

                if 1 <= t:
                    # ---- Phase B (z part): rest of cell1(t-1) z gate ----
                    mm_slot(pb_z1[:, 128:256], wz1, NK1, 2, e3b)  # z1.t3
                    pq_z1 = work.tile([128, 256], fp32, name="pq_z1")
                    nc.vector.tensor_sub(pq_z1[:, 0:128], t1c_z1[:], pb_z1[:, 0:128])
                    nc.vector.tensor_add(
                        pq_z1[:, 128:256], t1c_z1[:], pb_z1[:, 128:256]
                    )
                    z1_t = work.tile([128, 256], fp32, name="z1_t")
                    nc.scalar.activation(
                        z1_t[:, 0:128], pq_z1[:, 0:128], AF.Sigmoid,
                        bias=bias[:, 6:7],
                    )
                    nc.scalar.activation(
                        z1_t[:, 128:256], pq_z1[:, 128:256], AF.Sigmoid,
                        bias=bias[:, 7:8],
                    )
                    if t > 1:
                        w1_upd = update_pre(h1loc[prv], z1_t, "h1")

                    # ---- Phase D: cell1(t-1) candidate + update ----
                    e1d = hs_tiles(hs0[cur], 0)
                    e2d = h_i_tiles(h0f[cur], 0)
                    e3d = h_r_tiles(h0f[cur], 0)
                    if t > 1:
                        tmp1f = work.tile([128, H], bf16, name="tmp1f")
                        dma_gather_in(tmp1f, ag_t1)
                        ts1 = work.tile([128, M], bf16, name="ts1")
                        sum_ri(ts1, tmp1f)
                        e1d = e1d + hs_tiles(ts1, 8)
                        e2d = e2d + h_i_tiles(tmp1f, 8)
                        e3d = e3d + h_r_tiles(tmp1f, 8)
                    pb_h1 = ph_pool.tile([128, 256], fp32, name="pb_h1")
                    hh1_t = work.tile([128, 256], fp32, name="hh1_t")
                    emit_gate(
                        wh1, NK1, e1d, e2d, e3d, pth[:, 128:256], pb_h1, hh1_t,
                        10, AF.Tanh, "h1",
                    )
                    state_update(
                        w1_upd if t > 1 else None,
                        h1loc[cur], z1_t, hh1_t, first=(t == 1),
                    )
                    h1_sh = work.tile([128, 256], bf16, name="h1_sh")
                    nc.vector.tensor_copy(h1_sh[:], h1loc[cur][:])
                    pend_ag_h1 = ag_shard(h1_sh, "h1")

                if t < t_steps:
                    new_state_tiles(prv)  # rotate buffers for next slot

            # ---------------- final FC ----------------
            h1_final = spool.tile([128, H], bf16, name="h1_final")
            dma_gather_in(h1_final, pend_ag_h1)
            pfc = ph_pool.tile([128, 256], fp32, name="pth")
            for kt in range(NHT):
                nc.tensor.matmul(
                    pfc[:, 0:OUT],
                    h1_final[:, kt * 128 : (kt + 1) * 128],
                    wfc[:, kt * OUT : (kt + 1) * OUT],
                    start=(kt == 0),
                    stop=(kt == NHT - 1),
                )
            out_sb = work.tile([128, OUT], fp32, name="out_sb")
            nc.vector.tensor_add(out_sb[:], pfc[:, 0:OUT], fcbias[:])
            nc.sync.dma_start(d_out[:], out_sb[:])

    nc.compile()
    return nc


_CACHE = {}


def _get_nc(t_steps=T, no_collectives=False):
    key = (t_steps, no_collectives)
    if key not in _CACHE:
        _CACHE[key] = build_kernel(t_steps, no_collectives=no_collectives)
    return _CACHE[key]


def run(inputs, t_steps=T, trace=False):
    from concourse import bass_utils

    nc = _get_nc(t_steps)
    in_maps = host_prep(inputs)
    res = bass_utils.run_bass_kernel_spmd(
        nc, in_maps, core_ids=list(range(NCORES)), trace=trace
    )
    out = np.asarray(res.results[0]["out"], np.float32)
    return out, res


def timed_run(inputs, t_steps=T, iters=4, no_collectives=False, measure_reps=False):
    """Execute via PJRT with a persistent jitted executable; time each call."""
    import time

    import jax
    from jax.sharding import Mesh, PartitionSpec
    from jax.experimental.shard_map import shard_map

    import concourse.mybir as mybir
    from concourse import bass2jax

    nc = _get_nc(t_steps, no_collectives=no_collectives)
    in_maps = host_prep(inputs)
    bass2jax.install_neuronx_cc_hook()

    partition_name = nc.partition_id_tensor.name if nc.partition_id_tensor else None
    in_names, out_names, out_avals, zero_outs = [], [], [], []
    for alloc in nc.m.functions[0].allocations:
        if not isinstance(alloc, mybir.MemoryLocationSet):
            continue
        name = alloc.memorylocations[0].name
        if alloc.kind == "ExternalInput":
            if name != partition_name:
                in_names.append(name)
        elif alloc.kind == "ExternalOutput":
            out_names.append(name)
            shape = tuple(alloc.tensor_shape)
            dtype = mybir.dt.np(alloc.dtype)
            out_avals.append(jax.core.ShapedArray(shape, dtype))
            zero_outs.append(np.zeros(shape, dtype))
    n_params = len(in_names)
    n_outs = len(out_avals)
    all_in_names = list(in_names) + list(out_names)
    if partition_name is not None:
        all_in_names = all_in_names + [partition_name]

    def _body(*args):
        operands = list(args)
        if partition_name is not None:
            operands.append(bass2jax.partition_id_tensor())
        outs = bass2jax._bass_exec_p.bind(
            *operands,
            out_avals=tuple(out_avals),
            in_names=tuple(all_in_names),
            out_names=tuple(out_names),
            lowering_input_output_aliases=(),
            sim_require_finite=True,
            sim_require_nnan=True,
            nc=nc,
        )
        return tuple(outs)

    devices = jax.devices()[:NCORES]
    mesh = Mesh(np.asarray(devices), ("core",))
    in_specs = (PartitionSpec("core"),) * (n_params + n_outs)
    out_specs = (PartitionSpec("core"),) * n_outs
    donate = tuple(range(n_params, n_params + n_outs))
    sharded = jax.jit(
        shard_map(
            _body, mesh=mesh, in_specs=in_specs, out_specs=out_specs, check_rep=False
        ),
        donate_argnums=donate,
        keep_unused=True,
    )
    per_core = [[np.asarray(m[name]) for name in in_names] for m in in_maps]
    concat_in = [
        np.concatenate([per_core[c][i] for c in range(NCORES)], axis=0)
        for i in range(n_params)
    ]
    sharding = jax.sharding.NamedSharding(mesh, PartitionSpec("core"))
    dev_in = [jax.device_put(a, sharding) for a in concat_in]

    def one_call():
        zeros = [
            jax.device_put(
                np.zeros((NCORES * z.shape[0], *z.shape[1:]), z.dtype), sharding
            )
            for z in zero_outs
        ]
        for z in zeros:
            z.block_until_ready()
        t0 = time.perf_counter()
        outs = sharded(*dev_in, *zeros)
        for o in outs:
            o.block_until_ready()
        return time.perf_counter() - t0, outs

    times = []
    outs = None
    for _ in range(iters):
        dt, outs = one_call()
        times.append(dt)
    out0 = np.asarray(outs[0]).reshape(NCORES, *out_avals[0].shape)[0]

    return dict(times=times, best=min(times), out=np.asarray(out0, np.float32))


def kernel(**inputs):
    out, _ = run(inputs)
    return out


# revision 18
# speedup vs baseline: 1.1634x; 1.1634x over previous
"""Trainium2 Bass kernel for a 2-layer complex-gated GRU.

Problem (hardcoded):
  B=128, T=128, IN=256, H=2048, OUT=64, fp32.
  reference: 2 stacked complex GRU cells scanned over T, then a complex FC.

Strategy: 8-way tensor-parallel over the complex gate output dim (each core
owns 128 of the 1024 complex outputs per gate => 256 rows of the 2048-row
real-valued hidden state). Everything lives transposed ([features, batch]).

Each complex linear uses the 3-multiplication (Gauss) form:
  t1 = Wr   @ (cr+ci)
  t2 = (Wr+Wi) @ ci
  t3 = (Wi-Wr) @ cr
  out_r = t1 - t2,  out_i = t1 + t3
which is 25% fewer matmul FLOPs than the merged 2Kx2M real form. t1/t2/t3
accumulate in separate PSUM column slots; a DVE sub/add pair combines them,
then ScalarE applies sigmoid/tanh with the bias.

Per step, each cell needs two 8-core all-gathers: tmp = r (.) h (computed on
the owner core from its fp32 state shard BEFORE the gather - no full-width
multiply after), and h'. The two cells are software-pipelined (cell0 at t
with cell1 at t-1) so collective latency hides under the other cell's
matmuls. The r-gate is computed before the z-gate so its AG dispatches
early; the z-gate matmuls run under the AG. Matmul operands are bf16,
accumulation fp32 in PSUM, state update in fp32 from the core-local shard.
"""

import numpy as np
import ml_dtypes

B, T, IN, H, OUT = 128, 128, 256, 2048, 64
NCORES = 8
M = H // 2  # 1024 complex gate outputs per gate
SH = M // NCORES  # 128 complex outputs per core per gate
N0 = (IN + H) // 2  # 1152 complex contraction length (layer 0)
N1 = (H + H) // 2  # 2048 complex contraction length (layer 1)
NK0 = N0 // 128  # 9
NK1 = N1 // 128  # 16
NHT = H // 128  # 16 tiles of the hidden state

BF16 = ml_dtypes.bfloat16


def _sbuf_layout(w, ncols=128):
    """[K, ncols] -> [128, (K//128)*ncols] with k-tile k at cols [k*nc:(k+1)*nc]."""
    K = w.shape[0]
    nk = K // 128
    return np.ascontiguousarray(
        w.reshape(nk, 128, ncols).transpose(1, 0, 2).reshape(128, nk * ncols)
    )


def _gate_w(Wr, Wi, g, s):
    """Stationary weights for gate g, core s: [128, 3*nk*128] bf16.

    Slots (each nk k-tiles of [128k x 128out]):
      t1 = Wr (vs summed input), t2 = Wr+Wi (vs imag), t3 = Wi-Wr (vs real).
    Contraction rows are the natural complex input order (x first, then h
    complex components 0..1023 - which matches the gathered storage layout).
    """
    sl = slice(s * SH, (s + 1) * SH)
    a1 = Wr[g][sl]  # [128, n]
    a2 = (Wr[g] + Wi[g])[sl]
    a3 = (Wi[g] - Wr[g])[sl]
    blocks = [_sbuf_layout(a.T) for a in (a1, a2, a3)]
    return np.concatenate(blocks, axis=1)


def host_prep(inputs):
    """Build per-core in_maps (numpy) from the full problem inputs."""
    x = np.asarray(inputs["x"], np.float32)
    # xt[t] = [xr^T | xi^T | (xr+xi)^T], each [128, B]
    xr = x[:, :, :128]  # [B, T, 128]
    xi = x[:, :, 128:]
    xs = xr + xi
    xt = np.stack(
        [
            np.transpose(xr, (1, 2, 0)),  # [T, 128, B]
            np.transpose(xi, (1, 2, 0)),
            np.transpose(xs, (1, 2, 0)),
        ],
        axis=2,
    ).reshape(T, 128, 3 * B)
    xt = np.ascontiguousarray(xt).astype(BF16)

    # FC: out = [out_r | out_i] = h1c @ Wfc_big.T + [fcbr | fcbi]
    fcWr = np.asarray(inputs["fcWr"], np.float32)
    fcWi = np.asarray(inputs["fcWi"], np.float32)
    Wfc_big = np.block([[fcWr, -fcWi], [fcWi, fcWr]])  # [64, 2048] natural cols
    # storage layout: row p of h1f-tiles: shard s=p//256, q=p%256;
    # q<128 -> real comp s*128+q, q>=128 -> imag comp s*128+(q-128)
    p = np.arange(H)
    s_ = p // 256
    q = p % 256
    comp = s_ * 128 + (q % 128)
    h_perm = np.where(q < 128, comp, M + comp)
    Wfc_perm = Wfc_big[:, h_perm]
    wfc = np.ascontiguousarray(
        Wfc_perm.T.reshape(NHT, 128, OUT).transpose(1, 0, 2).reshape(128, NHT * OUT)
    ).astype(BF16)
    fcb = np.concatenate([inputs["fcbr"], inputs["fcbi"]]).astype(np.float32)
    fcbias = np.ascontiguousarray(np.broadcast_to(fcb, (128, OUT))).astype(np.float32)

    W0r = np.asarray(inputs["W0r"], np.float32)
    W0i = np.asarray(inputs["W0i"], np.float32)
    W1r = np.asarray(inputs["W1r"], np.float32)
    W1i = np.asarray(inputs["W1i"], np.float32)

    in_maps = []
    for s in range(NCORES):
        sl = slice(s * SH, (s + 1) * SH)
        bias = np.stack(
            [
                inputs["b0r"][0][sl], inputs["b0i"][0][sl],
                inputs["b0r"][1][sl], inputs["b0i"][1][sl],
                inputs["b0r"][2][sl], inputs["b0i"][2][sl],
                inputs["b1r"][0][sl], inputs["b1i"][0][sl],
                inputs["b1r"][1][sl], inputs["b1i"][1][sl],
                inputs["b1r"][2][sl], inputs["b1i"][2][sl],
            ],
            axis=1,
        ).astype(np.float32)  # [128, 12]
        in_maps.append(
            dict(
                xt=xt,
                wz0=_gate_w(W0r, W0i, 0, s).astype(BF16),
                wr0=_gate_w(W0r, W0i, 1, s).astype(BF16),
                wh0=_gate_w(W0r, W0i, 2, s).astype(BF16),
                wz1=_gate_w(W1r, W1i, 0, s).astype(BF16),
                wr1=_gate_w(W1r, W1i, 1, s).astype(BF16),
                wh1=_gate_w(W1r, W1i, 2, s).astype(BF16),
                wfc=wfc,
                bias=np.ascontiguousarray(bias),
                fcbias=fcbias,
            )
        )
    return in_maps


# ---------------------------------------------------------------------------
# numpy emulation of the sharded algorithm (host-side validation only)
# ---------------------------------------------------------------------------


def numpy_sharded_reference(inputs, t_steps=T):
    """Emulates the bass kernel's math in fp32 (no bf16 rounding)."""
    x = np.asarray(inputs["x"], np.float32)

    def sig(v):
        return 1.0 / (1.0 + np.exp(-v))

    def clin3(cr, ci, Wr, Wi, br, bi):
        # [n, B] inputs (natural complex order); Wr/Wi [1024, n]
        t1 = Wr @ (cr + ci)
        t2 = (Wr + Wi) @ ci
        t3 = (Wi - Wr) @ cr
        return t1 - t2 + br[:, None], t1 + t3 + bi[:, None]

    W0r = np.asarray(inputs["W0r"], np.float32)
    W0i = np.asarray(inputs["W0i"], np.float32)
    W1r = np.asarray(inputs["W1r"], np.float32)
    W1i = np.asarray(inputs["W1i"], np.float32)

    hr = [np.zeros((M, B), np.float32) for _ in range(2)]
    hi = [np.zeros((M, B), np.float32) for _ in range(2)]
    for t in range(t_steps):
        xr = x[:, t, :128].T
        xi = x[:, t, 128:].T
        for layer in range(2):
            Wr = (W0r, W1r)[layer]
            Wi = (W0i, W1i)[layer]
            br = (inputs["b0r"], inputs["b1r"])[layer]
            bi = (inputs["b0i"], inputs["b1i"])[layer]
            if layer == 0:
                cr = np.concatenate([xr, hr[0]], axis=0)
                ci = np.concatenate([xi, hi[0]], axis=0)
            else:
                cr = np.concatenate([hr[0], hr[1]], axis=0)
                ci = np.concatenate([hi[0], hi[1]], axis=0)
            zr, zi = clin3(cr, ci, Wr[0], Wi[0], br[0], bi[0])
            z_r, z_i = sig(zr), sig(zi)
            rr, ri = clin3(cr, ci, Wr[1], Wi[1], br[1], bi[1])
            r_r, r_i = sig(rr), sig(ri)
            tr = r_r * hr[layer]
            ti = r_i * hi[layer]
            if layer == 0:
                cr2 = np.concatenate([xr, tr], axis=0)
                ci2 = np.concatenate([xi, ti], axis=0)
            else:
                cr2 = np.concatenate([hr[0], tr], axis=0)
                ci2 = np.concatenate([hi[0], ti], axis=0)
            hhr, hhi = clin3(cr2, ci2, Wr[2], Wi[2], br[2], bi[2])
            hh_r, hh_i = np.tanh(hhr), np.tanh(hhi)
            hr[layer] = (1.0 - z_r) * hr[layer] + z_r * hh_r
            hi[layer] = (1.0 - z_i) * hi[layer] + z_i * hh_i
    out_r = np.asarray(inputs["fcWr"], np.float32) @ hr[1] \
        - np.asarray(inputs["fcWi"], np.float32) @ hi[1] \
        + np.asarray(inputs["fcbr"], np.float32)[:, None]
    out_i = np.asarray(inputs["fcWr"], np.float32) @ hi[1] \
        + np.asarray(inputs["fcWi"], np.float32) @ hr[1] \
        + np.asarray(inputs["fcbi"], np.float32)[:, None]
    return np.concatenate([out_r, out_i], axis=0).T


# ---------------------------------------------------------------------------
# bass kernel
# ---------------------------------------------------------------------------


def build_kernel(t_steps=T, no_collectives=False):
    import concourse.bacc as bacc
    import concourse.mybir as mybir
    import concourse.tile as tile

    fp32 = mybir.dt.float32
    bf16 = mybir.dt.bfloat16
    AF = mybir.ActivationFunctionType

    nc = bacc.Bacc(
        "TRN2", target_bir_lowering=False, debug=False, num_devices=NCORES
    )

    d_xt = nc.dram_tensor("xt", [T, 128, 3 * B], bf16, kind="ExternalInput")
    d_wz0 = nc.dram_tensor("wz0", [128, 3 * NK0 * 128], bf16, kind="ExternalInput")
    d_wr0 = nc.dram_tensor("wr0", [128, 3 * NK0 * 128], bf16, kind="ExternalInput")
    d_wh0 = nc.dram_tensor("wh0", [128, 3 * NK0 * 128], bf16, kind="ExternalInput")
    d_wz1 = nc.dram_tensor("wz1", [128, 3 * NK1 * 128], bf16, kind="ExternalInput")
    d_wr1 = nc.dram_tensor("wr1", [128, 3 * NK1 * 128], bf16, kind="ExternalInput")
    d_wh1 = nc.dram_tensor("wh1", [128, 3 * NK1 * 128], bf16, kind="ExternalInput")
    d_wfc = nc.dram_tensor("wfc", [128, NHT * OUT], bf16, kind="ExternalInput")
    d_bias = nc.dram_tensor("bias", [128, 12], fp32, kind="ExternalInput")
    d_fcbias = nc.dram_tensor("fcbias", [128, OUT], fp32, kind="ExternalInput")
    d_out = nc.dram_tensor("out", [B, OUT], fp32, kind="ExternalOutput")

    RG = [list(range(NCORES))]

    with tile.TileContext(nc) as tc:
        with (
            tc.tile_pool(name="wpool", bufs=1) as wpool,
            tc.tile_pool(name="state", bufs=1) as spool,
            tc.tile_pool(name="work", bufs=2) as work,
            tc.tile_pool(name="xpool", bufs=3) as xpool,
            tc.tile_pool(name="pzr", bufs=1, space="PSUM") as pzr_pool,
            tc.tile_pool(name="ph", bufs=1, space="PSUM") as ph_pool,
            tc.tile_pool(name="dram", bufs=2, space="DRAM") as dram,
        ):
            # --- persistent weights -> SBUF ---
            wz0 = wpool.tile([128, 3 * NK0 * 128], bf16)
            wr0 = wpool.tile([128, 3 * NK0 * 128], bf16)
            wh0 = wpool.tile([128, 3 * NK0 * 128], bf16)
            wz1 = wpool.tile([128, 3 * NK1 * 128], bf16)
            wr1 = wpool.tile([128, 3 * NK1 * 128], bf16)
            wh1 = wpool.tile([128, 3 * NK1 * 128], bf16)
            wfc = wpool.tile([128, NHT * OUT], bf16)
            bias = wpool.tile([128, 12], fp32)
            fcbias = wpool.tile([128, OUT], fp32)
            for dst, src in [
                (wz0, d_wz0), (wr0, d_wr0), (wh0, d_wh0),
                (wz1, d_wz1), (wr1, d_wr1), (wh1, d_wh1),
                (wfc, d_wfc), (bias, d_bias), (fcbias, d_fcbias),
            ]:
                nc.sync.dma_start(dst[:], src[:])

            def mm_slot(psum_ap, wsb, nk, slot, rhs_entries):
                """Accumulate one Gauss slot into psum_ap ([128, 128]).

                wsb: [128, 3*nk*128] stationary; slot in {0,1,2}; rhs_entries =
                [(k, rhs_ap)] with k the k-tile index within the slot."""
                last = len(rhs_entries) - 1
                base = slot * nk * 128
                for idx, (kg, rap) in enumerate(rhs_entries):
                    nc.tensor.matmul(
                        psum_ap,
                        wsb[:, base + kg * 128 : base + (kg + 1) * 128],
                        rap,
                        start=(idx == 0),
                        stop=(idx == last),
                    )

            def h_r_tiles(hf, base):
                # real-part k-tiles of a gathered state (storage layout)
                return [(base + i, hf[:, i * 256 : i * 256 + 128]) for i in range(8)]

            def h_i_tiles(hf, base):
                return [
                    (base + i, hf[:, i * 256 + 128 : i * 256 + 256]) for i in range(8)
                ]

            def hs_tiles(hs, base):
                # summed (r+i) k-tiles [128, 1024]
                return [(base + i, hs[:, i * 128 : (i + 1) * 128]) for i in range(8)]

            # state buffers (python-managed ping-pong)
            h0f = [None, None]  # full h0 (bf16, [128, H]) storage layout
            h1f = [None, None]
            hs0 = [None, None]  # summed h0 (r+i) [128, 1024] bf16
            hs1 = [None, None]
            h0loc = [None, None]  # own fp32 shard [128, 256]
            h1loc = [None, None]

            def new_state_tiles(i):
                h0f[i] = spool.tile([128, H], bf16, name=f"h0f{i}")
                h1f[i] = spool.tile([128, H], bf16, name=f"h1f{i}")
                hs0[i] = spool.tile([128, M], bf16, name=f"hs0{i}")
                hs1[i] = spool.tile([128, M], bf16, name=f"hs1{i}")
                h0loc[i] = spool.tile([128, 256], fp32, name=f"h0loc{i}")
                h1loc[i] = spool.tile([128, 256], fp32, name=f"h1loc{i}")

            new_state_tiles(0)
            new_state_tiles(1)

            def sum_ri(dst, hf):
                # dst[128, 1024] = hf real cols + imag cols (strided over shards)
                nc.vector.tensor_add(
                    dst[:].rearrange("p (s c) -> p s c", c=128),
                    hf[:].rearrange("p (s two c) -> p s two c", two=2, c=128)[
                        :, :, 0, :
                    ],
                    hf[:].rearrange("p (s two c) -> p s two c", two=2, c=128)[
                        :, :, 1, :
                    ],
                )

            def emit_gate(w, nk, e1, e2, e3, pa_t1, pb, out_t, bcol, af, tag):
                """One Gauss gate: t2 MMs (bank b), t1 MMs (bank a, then an
                early ScalarE copy to SBUF that overlaps t3), t3 MMs (bank b),
                DVE combine, ScalarE activation.

                pa_t1: [128,128] psum AP (t1); pb: [128,256] psum AP (t2|t3)."""
                mm_slot(pb[:, 0:128], w, nk, 1, e2)  # t2
                mm_slot(pa_t1, w, nk, 0, e1)  # t1
                t1c = work.tile([128, 128], fp32, name=f"t1c_{tag}")
                nc.scalar.copy(t1c[:], pa_t1)
                mm_slot(pb[:, 128:256], w, nk, 2, e3)  # t3
                pq = work.tile([128, 256], fp32, name=f"pq_{tag}")
                nc.vector.tensor_sub(pq[:, 0:128], t1c[:], pb[:, 0:128])
                nc.vector.tensor_add(pq[:, 128:256], t1c[:], pb[:, 128:256])
                nc.scalar.activation(
                    out_t[:, 0:128], pq[:, 0:128], af,
                    bias=bias[:, bcol : bcol + 1],
                )
                nc.scalar.activation(
                    out_t[:, 128:256], pq[:, 128:256], af,
                    bias=bias[:, bcol + 1 : bcol + 2],
                )

            def ag_shard(shard_bf, tag):
                """DMA shard [128, 256] -> DRAM [256, 128], AllGather, return
                the gathered DRAM tile [H, 128]."""
                agin = dram.tile([256, B], bf16, name=f"agin_{tag}")
                agout = dram.tile(
                    [H, B], bf16, name=f"agout_{tag}",
                    addr_space="Local" if no_collectives else "Shared",
                )
                nc.scalar.dma_start(
                    agin[:].rearrange("(a p) b -> p a b", p=128),
                    shard_bf[:].rearrange("p (a b) -> p a b", b=B),
                )
                if no_collectives:
                    for s in range(NCORES):
                        nc.sync.dma_start(
                            agout[s * 256 : (s + 1) * 256, :], agin[:]
                        )
                else:
                    nc.gpsimd.collective_compute(
                        "AllGather",
                        mybir.AluOpType.bypass,
                        replica_groups=RG,
                        ins=[agin[:]],
                        outs=[agout[:]],
                    )
                return agout

            def dma_gather_in(hf, agout):
                half = NHT // 2
                for i, eng in enumerate((nc.sync, nc.scalar)):
                    eng.dma_start(
                        hf[:, i * half * 128 : (i + 1) * half * 128].rearrange(
                            "p (n b) -> p n b", b=B
                        ),
                        agout[i * half * 128 : (i + 1) * half * 128, :].rearrange(
                            "(n p) b -> p n b", p=128
                        ),
                    )

            def update_pre(loc_prev, z_t, tag):
                """Off-critical-path part of h' = (1-z)h + z hh = (h - z h) + z hh."""
                u = work.tile([128, 256], fp32, name=f"upd_u_{tag}")
                w_ = work.tile([128, 256], fp32, name=f"upd_w_{tag}")
                nc.vector.tensor_mul(u[:], z_t[:], loc_prev[:])
                nc.vector.tensor_sub(w_[:], loc_prev[:], u[:])
                return w_

            def state_update(w_, loc_new, z_t, hh_t, first):
                if first:
                    nc.vector.tensor_mul(loc_new[:], z_t[:], hh_t[:])
                else:
                    v = work.tile([128, 256], fp32, name="upd_v")
                    nc.vector.tensor_mul(v[:], z_t[:], hh_t[:])
                    nc.vector.tensor_add(loc_new[:], w_[:], v[:])

            # ---------------- main pipelined loop ----------------
            # slot t: cell0(t) interleaved with cell1(t-1)
            pend_ag_h0 = None  # AG of h0'(t-1), lands in h0f[cur]
            pend_ag_h1 = None  # AG of h1'(t-2), lands in h1f[cur]

            for t in range(t_steps + 1):
                cur = t % 2
                prv = 1 - cur

                if pend_ag_h0 is not None:
                    dma_gather_in(h0f[cur], pend_ag_h0)
                    pend_ag_h0 = None
                    sum_ri(hs0[cur], h0f[cur])
                if pend_ag_h1 is not None:
                    dma_gather_in(h1f[cur], pend_ag_h1)
                    pend_ag_h1 = None
                    sum_ri(hs1[cur], h1f[cur])

                # psum tiles for this slot:
                #   pt1  [128,512]: z0.t1 | r0.t1 | z1.t1 | r1.t1   (1 bank)
                #   pb_* [128,256]: t2 | t3 per gate               (1 bank each)
                #   pth  [128,256]: c0.h.t1 | c1.h.t1              (1 bank)
                pt1 = pzr_pool.tile([128, 512], fp32, name="pt1")
                pth = ph_pool.tile([128, 256], fp32, name="pth")

                ag_t0 = None
                z0_t = None
                if t < t_steps:
                    # ---- Phase A: cell0(t) r gate then z gate ----
                    xt_t = xpool.tile([128, 3 * B], bf16, name="xt_t")
                    nc.sync.dma_start(xt_t[:], d_xt[t])
                    e1, e2, e3 = (
                        [(0, xt_t[:, 256:384])],
                        [(0, xt_t[:, 128:256])],
                        [(0, xt_t[:, 0:128])],
                    )
                    if t > 0:
                        e1 = e1 + hs_tiles(hs0[cur], 1)
                        e2 = e2 + h_i_tiles(h0f[cur], 1)
                        e3 = e3 + h_r_tiles(h0f[cur], 1)
                    pb_r0 = pzr_pool.tile([128, 256], fp32, name="pb_r0")
                    r0_t = work.tile([128, 256], fp32, name="r0_t")
                    emit_gate(
                        wr0, NK0, e1, e2, e3, pt1[:, 128:256], pb_r0, r0_t,
                        2, AF.Sigmoid, "r0",
                    )
                    tmp0_sh = work.tile([128, 256], bf16, name="tmp0_sh")
                    if t > 0:
                        nc.vector.tensor_mul(tmp0_sh[:], r0_t[:], h0loc[prv][:])
                        ag_t0 = ag_shard(tmp0_sh, "t0")
                    pb_z0 = pzr_pool.tile([128, 256], fp32, name="pb_z0")
                    z0_t = work.tile([128, 256], fp32, name="z0_t")
                    emit_gate(
                        wz0, NK0, e1, e2, e3, pt1[:, 0:128], pb_z0, z0_t,
                        0, AF.Sigmoid, "z0",
                    )
                    if t > 0:
                        w0_upd = update_pre(h0loc[prv], z0_t, "h0")

                z1_t = None
                if 1 <= t:
                    # ---- Phase B (r part): cell1(t-1) r gate ----
                    e1b = hs_tiles(hs0[cur], 0)
                    e2b = h_i_tiles(h0f[cur], 0)
                    e3b = h_r_tiles(h0f[cur], 0)
                    if t > 1:
                        e1b = e1b + hs_tiles(hs1[cur], 8)
                        e2b = e2b + h_i_tiles(h1f[cur], 8)
                        e3b = e3b + h_r_tiles(h1f[cur], 8)
                    pb_r1 = pzr_pool.tile([128, 256], fp32, name="pb_r1")
                    r1_t = work.tile([128, 256], fp32, name="r1_t")
                    emit_gate(
                        wr1, NK1, e1b, e2b, e3b, pt1[:, 384:512], pb_r1, r1_t,
                        8, AF.Sigmoid, "r1",
                    )
                    tmp1_sh = work.tile([128, 256], bf16, name="tmp1_sh")
                    ag_t1 = None
                    if t > 1:
                        nc.vector.tensor_mul(tmp1_sh[:], r1_t[:], h1loc[prv][:])
                        ag_t1 = ag_shard(tmp1_sh, "t1")

                if t < t_steps:
                    # ---- Phase C: cell0(t) candidate + update ----
                    e1, e2, e3 = (
                        [(0, xt_t[:, 256:384])],
                        [(0, xt_t[:, 128:256])],
                        [(0, xt_t[:, 0:128])],
                    )
                    if t > 0:
                        tmp0f = work.tile([128, H], bf16, name="tmp0f")
                        dma_gather_in(tmp0f, ag_t0)
                        ts0 = work.tile([128, M], bf16, name="ts0")
                        sum_ri(ts0, tmp0f)
                        e1 = e1 + hs_tiles(ts0, 1)
                        e2 = e2 + h_i_tiles(tmp0f, 1)
                        e3 = e3 + h_r_tiles(tmp0f, 1)
                    pb_h0 = ph_pool.tile([128, 256], fp32, name="pb_h0")
                    hh0_t = work.tile([128, 256], fp32, name="hh0_t")
                    emit_gate(
                        wh0, NK0, e1, e2, e3, pth[:, 0:128], pb_h0, hh0_t,
                        4, AF.Tanh, "h0",
                    )
                    state_update(
                        w0_upd if t > 0 else None,
                        h0loc[cur], z0_t, hh0_t, first=(t == 0),
                    )
                    h0_sh = work.tile([128, 256], bf16, name="h0_sh")
                    nc.vector.tensor_copy(h0_sh[:], h0loc[cur][:])
                    pend_ag_h0 = ag_shard(h0_sh, "h0")

                if 1 <= t:
                    # ---- Phase B (z part): cell1(t-1) z gate ----
                    pb_z1 = pzr_pool.tile([128, 256], fp32, name="pb_z1")
                    z1_t = work.tile([128, 256], fp32, name="z1_t")
                    emit_gate(
                        wz1, NK1, e1b, e2b, e3b, pt1[:, 256:384], pb_z1, z1_t,
                        6, AF.Sigmoid, "z1",
                    )
                    if t > 1:
                        w1_upd = update_pre(h1loc[prv], z1_t, "h1")

                    # ---- Phase D: cell1(t-1) candidate + update ----
                    e1d = hs_tiles(hs0[cur], 0)
                    e2d = h_i_tiles(h0f[cur], 0)
                    e3d = h_r_tiles(h0f[cur], 0)
                    if t > 1:
                        tmp1f = work.tile([128, H], bf16, name="tmp1f")
                        dma_gather_in(tmp1f, ag_t1)
                        ts1 = work.tile([128, M], bf16, name="ts1")
                        sum_ri(ts1, tmp1f)
                        e1d = e1d + hs_tiles(ts1, 8)
                        e2d = e2d + h_i_tiles(tmp1f, 8)
                        e3d = e3d + h_r_tiles(tmp1f, 8)
                    pb_h1 = ph_pool.tile([128, 256], fp32, name="pb_h1")
                    hh1_t = work.tile([128, 256], fp32, name="hh1_t")
                    emit_gate(
                        wh1, NK1, e1d, e2d, e3d, pth[:, 128:256], pb_h1, hh1_t,
                        10, AF.Tanh, "h1",
                    )
                    state_update(
                        w1_upd if t > 1 else None,
                        h1loc[cur], z1_t, hh1_t, first=(t == 1),
                    )
                    h1_sh = work.tile([128, 256], bf16, name="h1_sh")
                    nc.vector.tensor_copy(h1_sh[:], h1loc[cur][:])
                    pend_ag_h1 = ag_shard(h1_sh, "h1")

                if t < t_steps:
                    new_state_tiles(prv)  # rotate buffers for next slot

            # ---------------- final FC ----------------
            h1_final = spool.tile([128, H], bf16, name="h1_final")
            dma_gather_in(h1_final, pend_ag_h1)
            pfc = ph_pool.tile([128, 256], fp32, name="pth")
            for kt in range(NHT):
                nc.tensor.matmul(
                    pfc[:, 0:OUT],
                    h1_final[:, kt * 128 : (kt + 1) * 128],
                    wfc[:, kt * OUT : (kt + 1) * OUT],
                    start=(kt == 0),
                    stop=(kt == NHT - 1),
                )
            out_sb = work.tile([128, OUT], fp32, name="out_sb")
            nc.vector.tensor_add(out_sb[:], pfc[:, 0:OUT], fcbias[:])
            nc.sync.dma_start(d_out[:], out_sb[:])

    nc.compile()
    return nc


_CACHE = {}


def _get_nc(t_steps=T, no_collectives=False):
    key = (t_steps, no_collectives)
    if key not in _CACHE:
        _CACHE[key] = build_kernel(t_steps, no_collectives=no_collectives)
    return _CACHE[key]


def run(inputs, t_steps=T, trace=False):
    from concourse import bass_utils

    nc = _get_nc(t_steps)
    in_maps = host_prep(inputs)
    res = bass_utils.run_bass_kernel_spmd(
        nc, in_maps, core_ids=list(range(NCORES)), trace=trace
    )
    out = np.asarray(res.results[0]["out"], np.float32)
    return out, res


def timed_run(inputs, t_steps=T, iters=4, no_collectives=False, measure_reps=False):
    """Execute via PJRT with a persistent jitted executable; time each call."""
    import time

    import jax
    from jax.sharding import Mesh, PartitionSpec
    from jax.experimental.shard_map import shard_map

    import concourse.mybir as mybir
    from concourse import bass2jax

    nc = _get_nc(t_steps, no_collectives=no_collectives)
    in_maps = host_prep(inputs)
    bass2jax.install_neuronx_cc_hook()

    partition_name = nc.partition_id_tensor.name if nc.partition_id_tensor else None
    in_names, out_names, out_avals, zero_outs = [], [], [], []
    for alloc in nc.m.functions[0].allocations:
        if not isinstance(alloc, mybir.MemoryLocationSet):
            continue
        name = alloc.memorylocations[0].name
        if alloc.kind == "ExternalInput":
            if name != partition_name:
                in_names.append(name)
        elif alloc.kind == "ExternalOutput":
            out_names.append(name)
            shape = tuple(alloc.tensor_shape)
            dtype = mybir.dt.np(alloc.dtype)
            out_avals.append(jax.core.ShapedArray(shape, dtype))
            zero_outs.append(np.zeros(shape, dtype))
    n_params = len(in_names)
    n_outs = len(out_avals)
    all_in_names = list(in_names) + list(out_names)
    if partition_name is not None:
        all_in_names = all_in_names + [partition_name]

    def _body(*args):
        operands = list(args)
        if partition_name is not None:
            operands.append(bass2jax.partition_id_tensor())
        outs = bass2jax._bass_exec_p.bind(
            *operands,
            out_avals=tuple(out_avals),
            in_names=tuple(all_in_names),
            out_names=tuple(out_names),
            lowering_input_output_aliases=(),
            sim_require_finite=True,
            sim_require_nnan=True,
            nc=nc,
        )
        return tuple(outs)

    devices = jax.devices()[:NCORES]
    mesh = Mesh(np.asarray(devices), ("core",))
    in_specs = (PartitionSpec("core"),) * (n_params + n_outs)
    out_specs = (PartitionSpec("core"),) * n_outs
    donate = tuple(range(n_params, n_params + n_outs))
    sharded = jax.jit(
        shard_map(
            _body, mesh=mesh, in_specs=in_specs, out_specs=out_specs, check_rep=False
        ),
        donate_argnums=donate,
        keep_unused=True,
    )
    per_core = [[np.asarray(m[name]) for name in in_names] for m in in_maps]
    concat_in = [
        np.concatenate([per_core[c][i] for c in range(NCORES)], axis=0)
        for i in range(n_params)
    ]
    sharding = jax.sharding.NamedSharding(mesh, PartitionSpec("core"))
    dev_in = [jax.device_put(a, sharding) for a in concat_in]

    def one_call():
        zeros = [
            jax.device_put(
                np.zeros((NCORES * z.shape[0], *z.shape[1:]), z.dtype), sharding
            )
            for z in zero_outs
        ]
        for z in zeros:
            z.block_until_ready()
        t0 = time.perf_counter()
        outs = sharded(*dev_in, *zeros)
        for o in outs:
            o.block_until_ready()
        return time.perf_counter() - t0, outs

    times = []
    outs = None
    for _ in range(iters):
        dt, outs = one_call()
        times.append(dt)
    out0 = np.asarray(outs[0]).reshape(NCORES, *out_avals[0].shape)[0]

    return dict(times=times, best=min(times), out=np.asarray(out0, np.float32))


def kernel(**inputs):
    out, _ = run(inputs)
    return out
